# revision 26
# baseline (speedup 1.0000x reference)
"""Trainium2 Bass kernel for single-head attention (B=8, S=2048, DIN=768, DOUT=64).

Strategy: pure data parallelism — one batch element per NeuronCore (8 cores).
Per core, attention runs in transposed-score layout (no [S,S] transposes):

  qk1    [128, S]  = [Wq|Wk].T @ x.T     one fused projection (q rows 0-63,
                                         k rows 64-127) at full PE width
  dup    [128, S]  = [k; q]              partition-swapped copy of qk1 via
                                         SBUF->SBUF DMAs (cheaper than a 2nd
                                         projection pass)
  scT    [128, S]  = kT.T @ qT           2-way ROW-TILED: contraction is only
                                         64 (head dim), so two key tiles run
                                         CONCURRENTLY in PE row groups 0-63 /
                                         64-127 (tile_position from base
                                         partitions) -> ~2x score throughput
  e      = exp(scT / sqrt(S)) * keepT    exp on ScalarE, mask mul on VectorE
  ctxT   [65, S]  += v65.T @ e           v65 = [v | 1]; row 64 = softmax denom
  out              = (ctxT[:64]/ctxT[64]).T (PE transposes, psum pool shared
                                         with the projection chunks)

Scheduling notes (these are the difference between 128us and <60us):
  - The PE HAM clock-gate only unthrottles (1.2 -> 2.4 GHz) after ~3.4us of
    *sustained* PE activity. A dummy warmup matmul block runs while the xT
    DMAs land, so the projections and main loop run warm.
  - The 8.4 MB keep-mask DMA must be emitted AFTER xT and the dup copies, or
    its descriptors clog the rings and starve the critical path (a 17us PE
    stall in an earlier revision).
  - ctx matmuls are emitted one pair behind the score matmuls so the PE queue
    head never blocks on a not-yet-computed exp/mask tile.
  - The q loop is split in two halves (outer) so the ctx accumulator only
    needs [65, 1024] fp32 = 2 PSUM banks: 4 banks of double-buffered score
    pairs + 2 ctx + 2 shared proj/v/transpose chunks = 8/8.
"""

import math
import sys
from contextlib import ExitStack

import numpy as np

sys.path.insert(0, "/opt/trn_rl_repo")

import ml_dtypes  # noqa: E402

import concourse.bass as bass  # noqa: E402
import concourse.tile as tile  # noqa: E402
from concourse import bacc, mybir  # noqa: E402
from concourse.bass import ds  # noqa: E402
from concourse.bass_utils import run_bass_kernel_spmd  # noqa: E402
from concourse.masks import make_identity  # noqa: E402

B, S, DIN, DOUT = 8, 2048, 768, 64
P = 128
NCH = DIN // P  # 6 contraction chunks for the projections
KT = S // P  # 16 key tiles
NS = 512  # matmul moving-operand free dim (one PSUM bank fp32)
H = 2  # q halves (outer loop)
HQ = S // H  # 1024

F32 = mybir.dt.float32
F16 = mybir.dt.float16
BF16 = mybir.dt.bfloat16

_NC_CACHE = None


def build_nc():
    nc = bacc.Bacc("TRN2", target_bir_lowering=False, debug=False)

    xT = nc.declare_dram_parameter("xT", [DIN, S], BF16, isOutput=False)
    keep = nc.declare_dram_parameter("keep", [S, S], BF16, isOutput=False)
    # weights arrive pre-rearranged to [partition, chunk, col] on the host so
    # the DMA moves 128 fat rows instead of 768 tiny (256 B) descriptors
    wqk = nc.declare_dram_parameter("wqk", [P, NCH * P], BF16, isOutput=False)
    wv = nc.declare_dram_parameter("wv", [P, NCH * DOUT], BF16, isOutput=False)
    bqk = nc.declare_dram_parameter("bqk", [P, 1], F32, isOutput=False)
    out = nc.declare_dram_parameter("out", [P, KT * DOUT], F16, isOutput=True)

    inv_sqrt_s = float(1.0 / math.sqrt(S))

    with tile.TileContext(nc) as tc, ExitStack() as ctx:
        singles = ctx.enter_context(tc.tile_pool(name="singles", bufs=1))
        epool = ctx.enter_context(tc.tile_pool(name="epool", bufs=4))
        opool = ctx.enter_context(tc.tile_pool(name="opool", bufs=4))

        # ---- weights / bias (small, land fast)
        wqk_sb = singles.tile([P, NCH, P], BF16)
        nc.sync.dma_start(out=wqk_sb, in_=wqk.rearrange("p (c m) -> p c m", c=NCH))
        wv_sb = singles.tile([P, NCH, DOUT], BF16)
        nc.sync.dma_start(out=wv_sb, in_=wv.rearrange("p (c m) -> p c m", c=NCH))
        bqk_sb = singles.tile([P, 1], F32)
        nc.sync.dma_start(out=bqk_sb, in_=bqk[:, :])

        # ---- x.T resident (bf16), one trigger per chunk
        xT_sb = singles.tile([P, NCH, S], BF16)
        for c in range(NCH):
            nc.sync.dma_start(out=xT_sb[:, c, :], in_=xT[ds(c * P, P), :])

        keep_sb = singles.tile([P, KT, S], BF16)

        ident = singles.tile([P, P], F32)
        make_identity(nc, ident)

        # ---- v with a ones column: [s(128 part), ktile, 65] bf16
        v65_sb = singles.tile([P, KT, DOUT + 1], BF16)
        nc.gpsimd.memset(v65_sb, 1.0)

        qk1_sb = singles.tile([P, S], BF16)  # [q; k]
        dup_sb = singles.tile([P, S], BF16)  # [k; q]
        ctx_sb = singles.tile([DOUT + 1, S], F32)
        ostage = singles.tile([P, KT, DOUT], F16)

        # warm the exp activation table while the input DMAs land
        warm = singles.tile([P, 1], F32)
        nc.vector.memset(warm, 0.0)
        nc.scalar.activation(
            out=warm, in_=warm, func=mybir.ActivationFunctionType.Exp
        )

        with (
            tc.tile_pool(name="psS", bufs=2, space="PSUM") as psS,
            tc.tile_pool(name="psC", bufs=1, space="PSUM") as psC,
            tc.tile_pool(name="psV", bufs=2, space="PSUM") as psV,
        ):
            # ---- PE warmup: ~4us of dense dummy matmuls against the weight
            # tile (already resident) while xT streams in. This trips the HAM
            # activity monitor so everything after runs at 2.4 GHz.
            for w in range(10):
                wm_ps = psV.tile([P, NS], F32, tag="v")
                for r in range(4):
                    nc.tensor.matmul(
                        wm_ps[:, ds(r * P, P)],
                        lhsT=wqk_sb[:, w % NCH, :],
                        rhs=wqk_sb[:, (w + 1) % NCH, :],
                        start=True,
                        stop=True,
                    )

            # ---- fused q|k projection: qk1 = [Wq|Wk].T @ xT  (+bias).
            # Contraction-OUTER: chunk c's matmuls fire as soon as xT[c]
            # lands, so the projection finishes right behind the xT stream.
            # The 4 live accumulators borrow the (idle) score pool.
            p1a = psS.tile([P, HQ], F32, tag="sc")
            p1b = psS.tile([P, HQ], F32, tag="sc")

            def p1_mm(c, n):
                p1 = p1a if n < 2 else p1b
                nc.tensor.matmul(
                    p1[:, ds((n % 2) * NS, NS)],
                    lhsT=wqk_sb[:, c, :],
                    rhs=xT_sb[:, c, ds(n * NS, NS)],
                    start=(c == 0),
                    stop=(c == NCH - 1),
                )

            for c in range(NCH):
                for n in range(S // NS):
                    p1_mm(c, n)
            for n in range(S // NS):
                p1 = p1a if n < 2 else p1b
                nc.vector.tensor_scalar_add(
                    qk1_sb[:, ds(n * NS, NS)], p1[:, ds((n % 2) * NS, NS)], bqk_sb
                )
                # partition-swapped copy for the row-tiled score matmuls;
                # these sem-blocked triggers also hold the keep-mask bulk
                # back on the in-order sync queue so dup's descriptors
                # aren't stuck behind 8 MB of mask in the rings
                nc.sync.dma_start(
                    out=dup_sb[0:DOUT, ds(n * NS, NS)],
                    in_=qk1_sb[DOUT:P, ds(n * NS, NS)],
                )
                nc.sync.dma_start(
                    out=dup_sb[DOUT:P, ds(n * NS, NS)],
                    in_=qk1_sb[0:DOUT, ds(n * NS, NS)],
                )

            # keep mask in q-half order: h0 only reads columns 0-1023, so
            # its 4.2 MB streams first and every h0 deadline has ~10us of
            # margin; h1's half follows and is needed ~17us later
            for h in range(H):
                for t in range(KT):
                    nc.sync.dma_start(
                        out=keep_sb[:, t, ds(h * HQ, HQ)],
                        in_=keep[ds(t * P, P), ds(h * HQ, HQ)],
                    )

            # ---- v projection, groups of 4 key tiles per PSUM chunk; the
            # first groups fill the PE while dup/keep stream
            def emit_vproj4(g):
                v_ps = psV.tile([P, NS], F32, tag="v")
                for ti in range(4):
                    t = g * 4 + ti
                    for c in range(NCH):
                        nc.tensor.matmul(
                            v_ps[:, ds(ti * DOUT, DOUT)],
                            lhsT=xT_sb[:, c, ds(t * P, P)],
                            rhs=wv_sb[:, c, :],
                            start=(c == 0),
                            stop=(c == NCH - 1),
                        )
                nc.vector.tensor_copy(
                    v65_sb[:, ds(g * 4, 4), 0:DOUT],
                    v_ps[:, 0 : 4 * DOUT].rearrange("p (t m) -> p t m", m=DOUT),
                )

            # first 8 v-tiles up front (dense PE work while dup and the
            # first keep tiles stream); the rest interleave into early pairs
            emit_vproj4(0)
            emit_vproj4(1)

            # ---- main: outer loop over q halves, inner over key-tile pairs.
            # ctx matmuls run one pair behind the score matmuls so the PE
            # queue never stalls on exp/mask results. The h0 normalize is
            # deferred into h1's loop so it overlaps instead of stalling the
            # PE queue head at the half boundary.
            def emit_epilogue(h):
                # PE transposes share psV; out DMA per half
                for ti in range(HQ // P):
                    t = h * (HQ // P) + ti
                    tr = psV.tile([P, NS], F32, tag="v")
                    nc.tensor.transpose(
                        tr[:, 0 : DOUT + 1],
                        ctx_sb[:, ds(t * P, P)],
                        ident[0 : DOUT + 1, 0 : DOUT + 1],
                    )
                    rc = opool.tile([P, 1], F32, tag="rc")
                    nc.vector.reciprocal(rc, tr[:, DOUT : DOUT + 1])
                    nc.vector.tensor_scalar_mul(
                        ostage[:, t, :], tr[:, 0:DOUT], rc
                    )
                nc.sync.dma_start(
                    out=out[:, ds(h * (KT // 2) * DOUT, (KT // 2) * DOUT)],
                    in_=ostage[:, ds(h * (KT // 2), KT // 2), :].rearrange(
                        "p t m -> p (t m)"
                    ),
                )

            for h in range(H):
                ctx_ps = psC.tile([DOUT + 1, HQ], F32)
                pending = []  # (tile, ex) ready for the ctx matmul

                def flush_ctx():
                    for t, ex in pending:
                        for n in range(HQ // NS):
                            nc.tensor.matmul(
                                ctx_ps[:, ds(n * NS, NS)],
                                lhsT=v65_sb[:, t, :],
                                rhs=ex[:, ds(n * NS, NS)],
                                start=(t == 0),
                                stop=(t == KT - 1),
                            )
                    pending.clear()

                for j in range(KT // 2):
                    a, b = 2 * j, 2 * j + 1
                    sc_a = psS.tile([P, HQ], F32, tag="sc")
                    sc_b = psS.tile([P, HQ], F32, tag="sc")
                    # interleave row-group-0 / row-group-1 matmuls so both
                    # halves of the PE array stream concurrently
                    for n in range(HQ // NS):
                        nc.tensor.matmul(
                            sc_a[:, ds(n * NS, NS)],
                            lhsT=dup_sb[0:DOUT, ds(a * P, P)],
                            rhs=qk1_sb[0:DOUT, ds(h * HQ + n * NS, NS)],
                            start=True,
                            stop=True,
                        )
                        nc.tensor.matmul(
                            sc_b[:, ds(n * NS, NS)],
                            lhsT=qk1_sb[DOUT:P, ds(b * P, P)],
                            rhs=dup_sb[DOUT:P, ds(h * HQ + n * NS, NS)],
                            start=True,
                            stop=True,
                        )
                    flush_ctx()
                    for t, sc in ((a, sc_a), (b, sc_b)):
                        ex = epool.tile([P, HQ], BF16, tag="exp")
                        nc.scalar.activation(
                            out=ex,
                            in_=sc,
                            func=mybir.ActivationFunctionType.Exp,
                            scale=inv_sqrt_s,
                        )
                        nc.vector.tensor_mul(
                            ex, ex, keep_sb[:, t, ds(h * HQ, HQ)]
                        )
                        pending.append((t, ex))
                    if h == 0 and j < 2:
                        emit_vproj4(j + 2)
                    if h == 1 and j == 1:
                        emit_epilogue(0)
                flush_ctx()

                # flush this q half to SBUF
                nc.vector.tensor_copy(
                    ctx_sb[:, ds(h * HQ, HQ)], ctx_ps
                )
            emit_epilogue(1)

    nc.finalize()
    return nc


def _get_nc():
    global _NC_CACHE
    if _NC_CACHE is None:
        _NC_CACHE = build_nc()
    return _NC_CACHE


def kernel(**inputs):
    x = np.asarray(inputs["input_tensor"], dtype=np.float32)  # [B, S, DIN]
    mask = np.asarray(inputs["attention_mask"])  # [B, S, S] bool
    Wq = np.asarray(inputs["Wq"], dtype=np.float32)
    Wk = np.asarray(inputs["Wk"], dtype=np.float32)
    Wv = np.asarray(inputs["Wv"], dtype=np.float32)
    bq = np.asarray(inputs["bq"], dtype=np.float32)
    bk = np.asarray(inputs["bk"], dtype=np.float32)
    bv = np.asarray(inputs["bv"], dtype=np.float32)

    # pre-rearrange weights to [partition, chunk*col] so the device DMA is
    # 128 fat rows instead of 768 tiny descriptors
    wqk_b = np.ascontiguousarray(
        np.concatenate([Wq, Wk], axis=1)
        .reshape(NCH, P, P)
        .transpose(1, 0, 2)
        .reshape(P, NCH * P)
    ).astype(ml_dtypes.bfloat16)
    wv_b = np.ascontiguousarray(
        Wv.reshape(NCH, P, DOUT).transpose(1, 0, 2).reshape(P, NCH * DOUT)
    ).astype(ml_dtypes.bfloat16)
    bqk_c = np.ascontiguousarray(
        np.concatenate([bq, bk]).reshape(P, 1).astype(np.float32)
    )

    in_maps = []
    for b in range(B):
        xTb = np.ascontiguousarray(x[b].T).astype(ml_dtypes.bfloat16)  # [DIN, S]
        keepb = (~mask[b]).T.astype(ml_dtypes.bfloat16)  # [S, S] (k, q), 1=keep
        in_maps.append(
            {
                "xT": xTb,
                "keep": np.ascontiguousarray(keepb),
                "wqk": wqk_b,
                "wv": wv_b,
                "bqk": bqk_c,
            }
        )

    nc = _get_nc()
    res = run_bass_kernel_spmd(nc, in_maps, core_ids=list(range(B)))
    outs = []
    for b in range(B):
        ob = np.asarray(res.results[b]["out"], np.float32)  # [128, KT*DOUT]
        # out[p, t, d] = ctx[q = t*128 + p, d]
        ob = ob.reshape(P, KT, DOUT).transpose(1, 0, 2).reshape(S, DOUT)
        outs.append(ob)
    out = np.stack(outs) + bv[None, None, :]
    return out.astype(np.float32)
